# revision 51
# baseline (speedup 1.0000x reference)
"""BinMNIST binary-MLP forward pass on 8 Trainium2 NeuronCores.

Strategy (data-parallel batch 8 x 2048; weight-prep sharded 8-ways):
  - Activations feature-major in SBUF: [128 feat partitions, batch free].
  - Layer 1: x split into 2 exact fp16 terms (hi/lo, ~22 mantissa bits);
    sign(W1) is exact in fp16, so 2 fp16 matmuls with fp32 PSUM
    accumulation reproduce the fp32 matmul to ~2^-23 relative.
  - Layers 2-4 exact: inputs {-1,0,+1}, weights sign() to +-1; fp8e4
    products exact, fp32 PSUM accumulation, DoubleRow perf mode.
  - W2/W3 prep is sharded: each core receives a 512-row shard of W2/W3
    (input distribution), signs + transposes + fp8-casts its shard, and
    one combined AllGather replicates the transposed panels.
  - ALL transposes run on the PE (is_transpose matmuls through PSUM):
    the tile scheduler serializes xbar DMA transposes against
    collectives, so any xbar use would stall the whole kernel around
    the 320us panel AllGather. PE transposes are exempt.
  - BatchNorm (training mode, full-batch stats) + sign() folds into a
    per-feature threshold; per-core sums/sumsq combine with ~32KB
    AllReduces; boundary 1 is split (early kt signed mid-L1 into arena
    tiles carved from dead prep buffers) and the next layer's matmuls
    chase the sign stream down the k-tiles.
  - The last 7-9 feature tiles of h per layer stay SBUF-resident in
    reused prep regions, skipping their DRAM round trip.
"""

import numpy as np

import concourse.bass as bass
import concourse.mybir as mybir
import concourse.tile as tile
from concourse import bacc
from concourse.masks import make_identity

dt = mybir.dt
AF = mybir.ActivationFunctionType
ALU = mybir.AluOpType

N_CORES = 8
B = 16384
B_LOC = B // N_CORES          # 2048
IN_F = 784
K1P = 896                     # 784 padded to 7*128
KT1 = 7
H = 4096
NT = H // 128                 # 32 feature tiles
WSH = H // N_CORES            # 512-row weight shard per core
OUT_C = 10
BC = 512                      # batch chunk (one PSUM bank)
NBC = B_LOC // BC             # 4
BT = B_LOC // 128             # 16 batch tiles of 128
HB = B_LOC // 2               # 1024 (half-batch h chunk)
EPS = 1e-4
INV_B = 1.0 / float(B)

_CACHE = {}
_USE_CC = [True]


def _emit_stats_range(nc, pools, bn_all, g_vec, be_vec, cc_in, cc_out,
                      k0, k1, sig, bias_s):
    """bn_aggr -> sums/sumsq -> AllReduce -> threshold for kt in [k0, k1)."""
    sm = pools["small"]
    nk = k1 - k0
    mv = sm.tile([128, NT, 2], dt.float32, tag="mv", name="mv")
    for n in range(k0, k1):
        nc.vector.bn_aggr(mv[:, n, :], bn_all[:, n, :, :])
    sums = sm.tile([128, NT], dt.float32, tag="sums", name="sums")
    sumsq = sm.tile([128, NT], dt.float32, tag="sumsq", name="sumsq")
    # sum = mean * B_LOC ; sumsq = (var + mean^2) * B_LOC
    nc.vector.tensor_scalar_mul(sums[:, k0:k1], mv[:, k0:k1, 0:1],
                                float(B_LOC))
    tmp = sm.tile([128, NT], dt.float32, tag="tmp", name="tmp")
    nc.vector.tensor_mul(tmp[:, k0:k1], mv[:, k0:k1, 0:1], mv[:, k0:k1, 0:1])
    nc.vector.tensor_add(tmp[:, k0:k1], tmp[:, k0:k1], mv[:, k0:k1, 1:2])
    nc.vector.tensor_scalar_mul(sumsq[:, k0:k1], tmp[:, k0:k1], float(B_LOC))

    nc.sync.dma_start(cc_in[:, 0:nk], sums[:, k0:k1])
    nc.sync.dma_start(cc_in[:, nk : 2 * nk], sumsq[:, k0:k1])
    if _USE_CC[0]:
        nc.gpsimd.collective_compute(
            "AllReduce",
            ALU.add,
            replica_groups=[list(range(N_CORES))],
            ins=[cc_in.opt()],
            outs=[cc_out.opt()],
        )
    else:
        nc.sync.dma_start(cc_out[:], cc_in[:])
    gst = sm.tile([128, 2 * NT], dt.float32, tag="gst", name="gst")
    nc.sync.dma_start(gst[:, 0 : 2 * nk], cc_out[:])

    m = sm.tile([128, NT], dt.float32, tag="m", name="m")
    nc.vector.tensor_scalar_mul(m[:, k0:k1], gst[:, 0:nk], INV_B)
    v = sm.tile([128, NT], dt.float32, tag="v", name="v")
    nc.vector.tensor_scalar_mul(v[:, k0:k1], gst[:, nk : 2 * nk], INV_B)
    mm2 = sm.tile([128, NT], dt.float32, tag="mm2", name="mm2")
    nc.vector.tensor_mul(mm2[:, k0:k1], m[:, k0:k1], m[:, k0:k1])
    nc.vector.tensor_sub(v[:, k0:k1], v[:, k0:k1], mm2[:, k0:k1])
    nc.vector.tensor_scalar_add(v[:, k0:k1], v[:, k0:k1], EPS)
    sd = sm.tile([128, NT], dt.float32, tag="sd", name="sd")
    nc.scalar.activation(sd[:, k0:k1], v[:, k0:k1], AF.Sqrt)
    ginv = sm.tile([128, NT], dt.float32, tag="ginv", name="ginv")
    nc.vector.reciprocal(ginv[:, k0:k1], g_vec[:, k0:k1])
    # negT = be*sd/g - m ; sig = sign(g) ; bias = negT * sig
    t1 = sm.tile([128, NT], dt.float32, tag="t1", name="t1")
    nc.vector.tensor_mul(t1[:, k0:k1], be_vec[:, k0:k1], sd[:, k0:k1])
    nc.vector.tensor_mul(t1[:, k0:k1], t1[:, k0:k1], ginv[:, k0:k1])
    nc.vector.tensor_sub(t1[:, k0:k1], t1[:, k0:k1], m[:, k0:k1])
    nc.scalar.activation(sig[:, k0:k1], g_vec[:, k0:k1], AF.Sign)
    nc.vector.tensor_mul(bias_s[:, k0:k1], t1[:, k0:k1], sig[:, k0:k1])


def _sign_kt_closures(nc, pools, h_dram, s_dst, sig, bias_s, k0, k1):
    """Per-kt closures: h read (SP) + ACT Sign into s_dst(kt)."""
    hp = pools["hq"]

    def make(kt):
        def emit():
            for hh in range(2):
                h2 = hp.tile([128, HB], dt.float32, tag="hbig", name="h2")
                nc.sync.dma_start(
                    h2[:], h_dram[kt, :, hh * HB : (hh + 1) * HB]
                )
                dst, lk = s_dst(kt)
                nc.scalar.activation(
                    dst[:, lk, hh * HB : (hh + 1) * HB],
                    h2[:],
                    AF.Sign,
                    bias=bias_s[:, kt : kt + 1],
                    scale=sig[:, kt : kt + 1],
                )
        return emit

    return [make(kt) for kt in range(k0, k1)]


def _build(use_cc=True):
    _USE_CC[0] = use_cc
    nc = bacc.Bacc("TRN2", target_bir_lowering=False, debug=False,
                   num_devices=N_CORES if use_cc else 1)

    x_p = nc.dram_tensor("x", [B_LOC, IN_F], dt.float32, kind="ExternalInput")
    w1_p = nc.dram_tensor("W1", [H, IN_F], dt.float32, kind="ExternalInput")
    w2_p = nc.dram_tensor("W2", [WSH, H], dt.float32, kind="ExternalInput")
    w3_p = nc.dram_tensor("W3", [WSH, H], dt.float32, kind="ExternalInput")
    w4_p = nc.dram_tensor("W4", [OUT_C, H], dt.float32, kind="ExternalInput")
    vec_p = {}
    for name, n in [("b1", H), ("g1", H), ("be1", H), ("b2", H), ("g2", H),
                    ("be2", H), ("b3", H), ("g3", H), ("be3", H),
                    ("b4", OUT_C)]:
        vec_p[name] = nc.dram_tensor(name, [n], dt.float32,
                                     kind="ExternalInput")
    out_p = nc.dram_tensor("out", [B_LOC, OUT_C], dt.float32,
                           kind="ExternalOutput")

    with tile.TileContext(nc) as tc:
        with (
            tc.tile_pool(name="const", bufs=1) as constp,
            tc.tile_pool(name="small", bufs=1) as smallp,
            tc.tile_pool(name="xprep", bufs=2) as xprepp,
            tc.tile_pool(name="xprep1", bufs=2) as xprep1p,
            tc.tile_pool(name="w1prep", bufs=2) as w1prepp,
            tc.tile_pool(name="wprep", bufs=1) as wprepp,
            tc.tile_pool(name="xq", bufs=1) as xqp,
            tc.tile_pool(name="wts", bufs=3) as wtsp,
            tc.tile_pool(name="wpan", bufs=2) as wpanp,
            tc.tile_pool(name="hq", bufs=2) as hqp,
            tc.tile_pool(name="sres", bufs=1) as sresp,
            tc.tile_pool(name="l4", bufs=1) as l4p,
            tc.tile_pool(name="soft", bufs=2) as softp,
            tc.tile_pool(name="psum", bufs=8, space="PSUM") as psump,
            tc.tile_pool(name="dram", bufs=1, space="DRAM") as dramp,
        ):
            pools = {"small": smallp, "hq": hqp, "wprep": wprepp}

            # ---------- constants ----------
            id_sb = constp.tile([128, 128], dt.float32, tag="id", name="id_sb")
            make_identity(nc, id_sb[:])
            id16 = constp.tile([128, 128], dt.float16, tag="id16",
                               name="id16")
            nc.vector.tensor_copy(id16[:], id_sb[:])
            idb = constp.tile([128, 128], dt.bfloat16, tag="idb", name="idb")
            nc.vector.tensor_copy(idb[:], id_sb[:])

            # per-feature vectors -> [128, 32] via DVE 32x32 block transposes
            vecs = {}
            for name in ["b1", "g1", "be1", "b2", "g2", "be2", "b3", "g3",
                         "be3"]:
                vl = smallp.tile([32, 128], dt.float32, tag="vl",
                                 name=f"vl_{name}")
                nc.sync.dma_start(
                    vl[:], vec_p[name][:].rearrange("(t p) -> t p", p=128)
                )
                vt = constp.tile([128, 32], dt.float32, tag=f"vt_{name}",
                                 name=f"vt_{name}")
                for j in range(4):
                    nc.vector.transpose(
                        vt[j * 32 : (j + 1) * 32, 0:32],
                        vl[0:32, j * 32 : (j + 1) * 32],
                    )
                vecs[name] = vt
            b4sb = constp.tile([OUT_C, 1], dt.float32, tag="b4", name="b4sb")
            nc.sync.dma_start(
                b4sb[:], vec_p["b4"][:].rearrange("(n one) -> n one", one=1)
            )

            # ---------- DRAM scratch ----------
            h1_d = dramp.tile([NT, 128, B_LOC], dt.float32, tag="h1",
                              name="h1_d")
            h2_d = dramp.tile([NT, 128, B_LOC], dt.float32, tag="h2",
                              name="h2_d")
            h3_d = dramp.tile([NT, 128, B_LOC], dt.float32, tag="h3",
                              name="h3_d")
            ccs = {}
            for l in range(3):
                for h, nk in ((0, NT), (1, 14), (2, NT - 14), (3, 16),
                              (4, NT - 16)):
                    ccs[(l, h)] = (
                        dramp.tile([128, 2 * nk], dt.float32,
                                   tag=f"cci{l}_{h}", name=f"cc_in{l}_{h}"),
                        dramp.tile([128, 2 * nk], dt.float32,
                                   tag=f"cco{l}_{h}", name=f"cc_out{l}_{h}"),
                    )
            # sharded W2/W3 panel gather buffers (fp8, transposed layout)
            ccw_in = dramp.tile([2, 4, 128, NT, 128], dt.float8e4,
                                tag="ccwi", name="ccw_in")
            ccw_out = dramp.tile([N_CORES, 2, 4, 128, NT, 128], dt.float8e4,
                                 tag="ccwo", name="ccw_out")

            # s: current sign activations, fp8, feature-major
            s_tile = sresp.tile([128, NT, B_LOC], dt.float8e4, tag="s",
                                name="s_tile")

            # ---------- x: load + fp16 hi/lo split + transpose ----------
            xq = xqp.tile([128, KT1, 2, B_LOC], dt.float16, tag="xq",
                          name="xq")

            # pad region (k 784..895) of xq is zero for every batch tile;
            # the kt=6 transposes then overwrite the valid rows 0..15
            nc.vector.memset(xq[:, 6, :, :], 0.0)

            def emit_x_chunk(bt):
                xn = xprep1p.tile([128, IN_F], dt.float32, tag="xn",
                                  name="xn")
                nc.sync.dma_start(xn[:], x_p[bt * 128 : (bt + 1) * 128, :])
                bs = slice(bt * 128, (bt + 1) * 128)
                for kt in range(KT1):
                    kw = min(128, IN_F - kt * 128)
                    tp = psump.tile([128, 128], dt.float32, tag="ps",
                                    name="tpx")
                    nc.tensor.transpose(
                        tp[0:kw, :], xn[:, kt * 128 : kt * 128 + kw],
                        id_sb[:],
                    )
                    nc.vector.tensor_copy(xq[0:kw, kt, 0, bs], tp[0:kw, :])
                    xr = xprep1p.tile([128, 128], dt.float32, tag="xr",
                                      name="xr")
                    nc.vector.tensor_sub(xr[0:kw, :], tp[0:kw, :],
                                         xq[0:kw, kt, 0, bs])
                    nc.vector.tensor_copy(xq[0:kw, kt, 1, bs], xr[0:kw, :])

            # ---------- W1: sign(fp16) + transpose -> DRAM k-major ------
            # (transposes stay in phase 0: the scheduler serializes DMA
            # transposes with collectives, so none may overlap the gather)
            wts_tiles = {}

            def emit_w1_chunk(nt):
                w1f = w1prepp.tile([128, IN_F], dt.float32, tag="w1f",
                                   name="w1f", bufs=2)
                nc.sync.dma_start(w1f[:], w1_p[nt * 128 : (nt + 1) * 128, :])
                w1h = w1prepp.tile([128, K1P], dt.float16, tag="w1h",
                                   name="w1h", bufs=2)
                nc.scalar.memzero(w1h[:, IN_F:K1P])
                nc.scalar.activation(w1h[:, 0:IN_F], w1f[:], AF.Sign)
                wts = wtsp.tile([128, KT1, 128], dt.float16, tag="wts",
                                name="wts")
                tpa = psump.tile([128, 512], dt.float16, tag="ps", name="tpa")
                for j in range(4):
                    nc.tensor.transpose(
                        tpa[:, j * 128 : (j + 1) * 128],
                        w1h[:, j * 128 : (j + 1) * 128], id16[:],
                    )
                nc.vector.tensor_copy(
                    wts[:, 0:4, :].rearrange("p a b -> p (a b)"), tpa[:]
                )
                tpb = psump.tile([128, 384], dt.float16, tag="ps", name="tpb")
                for j in range(3):
                    nc.tensor.transpose(
                        tpb[:, j * 128 : (j + 1) * 128],
                        w1h[:, (4 + j) * 128 : (5 + j) * 128], id16[:],
                    )
                nc.vector.tensor_copy(
                    wts[:, 4:7, :].rearrange("p a b -> p (a b)"), tpb[:]
                )
                wts_tiles[nt] = wts


            # ---------- W2/W3 shard prep closures (+ one AllGather) --------
            HK = H // 4

            def make_w23(wi, wp_param, nb):
                def emit():
                    wb = wprepp.tile([128, H], dt.bfloat16, tag="wb",
                                     name="wb")
                    for kh in range(4):
                        wf = wprepp.tile([128, HK], dt.float32, tag="wf",
                                         name="wf", bufs=2)
                        nc.sync.dma_start(
                            wf[:],
                            wp_param[nb * 128 : (nb + 1) * 128,
                                     kh * HK : (kh + 1) * HK],
                        )
                        nc.scalar.activation(
                            wb[:, kh * HK : (kh + 1) * HK], wf[:], AF.Sign
                        )
                    wq = wprepp.tile([128, NT, 128], dt.float8e4, tag="wq",
                                     name="wq")
                    for g in range(8):
                        tpw = psump.tile([128, 512], dt.bfloat16, tag="ps",
                                         name="tpw")
                        for j in range(4):
                            kt = g * 4 + j
                            nc.tensor.transpose(
                                tpw[:, j * 128 : (j + 1) * 128],
                                wb[:, kt * 128 : (kt + 1) * 128], idb[:],
                            )
                        nc.vector.tensor_copy(
                            wq[:, g * 4 : (g + 1) * 4, :].rearrange(
                                "p a b -> p (a b)"
                            ),
                            tpw[:],
                        )
                    nc.sync.dma_start(ccw_in[wi, nb], wq[:])
                return emit

            def emit_gather():
                if _USE_CC[0]:
                    nc.gpsimd.collective_compute(
                        "AllGather",
                        ALU.bypass,
                        replica_groups=[list(range(N_CORES))],
                        ins=[ccw_in.opt()],
                        outs=[ccw_out.opt()],
                    )
                else:
                    nc.sync.dma_start(ccw_out[0], ccw_in[:])

            # round-robin the three prep streams; W23 front-loaded so the
            # gather's inputs complete as early as possible
            w23_q = [make_w23(wi, wp, nb)
                     for wi, wp in ((0, w2_p), (1, w3_p))
                     for nb in range(4)]
            x_q = [lambda bt=bt: emit_x_chunk(bt) for bt in range(BT)]
            for ch in x_q + w23_q:
                ch()
            for nt in range(2):
                emit_w1_chunk(nt)


            # ---------- W4: sign + transpose (PE) -> resident fp8 ----------
            wt4 = constp.tile([128, NT, 16], dt.float8e4, tag="wt4",
                              name="wt4")
            nc.vector.memset(wt4[:], 0.0)
            for kt in range(NT):
                w4c = w1prepp.tile([OUT_C, 128], dt.float32, tag="w4c",
                                   name="w4c")
                nc.sync.dma_start(w4c[:], w4_p[:, kt * 128 : (kt + 1) * 128])
                tp = psump.tile([128, 128], dt.float32, tag="ps", name="tp4")
                nc.tensor.transpose(tp[:], w4c[:], id_sb[0:OUT_C, :])
                nc.scalar.activation(wt4[:, kt, 0:OUT_C], tp[:, 0:OUT_C],
                                     AF.Sign)


            def make_arena_a():
                # 14 kt of fp8 [128, 2048] carved from dead wprep buffers
                tiles = [
                    (wprepp.tile([128, 4, B_LOC], dt.float8e4, tag="wb",
                                 name="arena_wb"), 4),
                    (wprepp.tile([128, 4, B_LOC], dt.float8e4, tag="wtp",
                                 name="arena_wtp"), 4),
                    (wprepp.tile([128, 2, B_LOC], dt.float8e4, tag="wq",
                                 name="arena_wq"), 2),
                    (wprepp.tile([128, 2, B_LOC], dt.float8e4, tag="wf",
                                 name="arena_wf0", bufs=2), 2),
                    (wprepp.tile([128, 2, B_LOC], dt.float8e4, tag="wf",
                                 name="arena_wf1", bufs=2), 2),
                ]

                def dst(kt):
                    base = 0
                    for t, n in tiles:
                        if kt < base + n:
                            return t, kt - base
                        base += n
                    raise AssertionError(kt)
                return dst

            def make_arena_b():
                t = xqp.tile([128, 16, B_LOC], dt.float8e4, tag="xq",
                             name="arena_b")
                return lambda kt: (t, kt)

            def make_arena_h(layer):
                # L2 may not touch wb/wtp: boundary-1's sign arena lives
                # there and L2's matmuls read it until the layer ends.
                big = xqp.tile([128, 7, B_LOC], dt.float32, tag="xq",
                               name=f"hres_big{layer}")
                extra = []
                if layer == 3:
                    extra = [
                        wprepp.tile([128, B_LOC], dt.float32, tag="wb",
                                    name="hres_wb3"),
                        wprepp.tile([128, B_LOC], dt.float32, tag="wtp",
                                    name="hres_wtp3"),
                    ]

                def at(j):
                    if j < 7:
                        return big[:, j, :]
                    return extra[j - 7][:]
                return at, 7 + len(extra)

            # s source per layer: (chooser, split point). None = all s_tile.
            layer_src = {2: (None, 0), 3: (None, 0), 4: (None, 0)}

            def rhs(layer, kt, bc):
                dst, split = layer_src[layer]
                if dst is None or kt >= split:
                    return s_tile[:, kt : kt + 2, bc * BC : (bc + 1) * BC]
                t, lk = dst(kt)
                return t[:, lk : lk + 2, bc * BC : (bc + 1) * BC]


            # ---------- layer 1: fp16 2-term, bc-inner ----------
            bn_all1 = smallp.tile([128, NT, NBC, 6], dt.float32, tag="bn1",
                                  name="bn_all1")
            sig1 = smallp.tile([128, NT], dt.float32, tag="sig", name="sig1")
            bia1 = smallp.tile([128, NT], dt.float32, tag="bias", name="bia1")
            b1_arena = None
            b1_signs = []
            for nt in range(NT):
                if nt + 2 < NT:
                    emit_w1_chunk(nt + 2)
                wts = wts_tiles[nt]
                pss = [
                    psump.tile([128, BC], dt.float32, tag="ps",
                               name=f"ps1_{bc}")
                    for bc in range(NBC)
                ]
                for kt in range(KT1):
                    for t in range(2):
                        for bc in range(NBC):
                            nc.tensor.matmul(
                                pss[bc][:],
                                wts[:, kt, :],
                                xq[:, kt, t, bc * BC : (bc + 1) * BC],
                                start=(kt == 0 and t == 0),
                                stop=(kt == KT1 - 1 and t == 1),
                            )
                for hh in range(2):
                    ht = hqp.tile([128, HB], dt.float32, tag="hbig",
                                  name="ht1")
                    for b2 in range(2):
                        bc = hh * 2 + b2
                        nc.vector.tensor_scalar(
                            ht[:, b2 * BC : (b2 + 1) * BC], pss[bc][:],
                            vecs["b1"][:, nt : nt + 1], 0.0,
                            op0=ALU.add, op1=ALU.max,
                        )
                        nc.vector.bn_stats(
                            bn_all1[:, nt, bc, :],
                            ht[:, b2 * BC : (b2 + 1) * BC],
                        )
                    nc.sync.dma_start(
                        h1_d[nt, :, hh * HB : (hh + 1) * HB], ht[:]
                    )
                if nt == 15:
                    b1_arena = make_arena_a()
                    _emit_stats_range(nc, pools, bn_all1, vecs["g1"],
                                      vecs["be1"], ccs[(0, 1)][0],
                                      ccs[(0, 1)][1], 0, 14, sig1, bia1)
                    b1_signs = _sign_kt_closures(
                        nc, pools, h1_d, lambda kt: b1_arena(kt),
                        sig1, bia1, 0, 14,
                    )
                if nt >= 29:
                    for _ in range(5):
                        if b1_signs:
                            b1_signs.pop(0)()

            while b1_signs:
                b1_signs.pop(0)()
            emit_gather()
            _emit_stats_range(nc, pools, bn_all1, vecs["g1"], vecs["be1"],
                              ccs[(0, 2)][0], ccs[(0, 2)][1], 14, NT,
                              sig1, bia1)
            for ch in _sign_kt_closures(nc, pools, h1_d,
                                        lambda kt: (s_tile, kt),
                                        sig1, bia1, 14, NT):
                ch()
            layer_src[2] = (b1_arena, 14)

            # ---------- layers 2 and 3: fp8 DoubleRow ----------
            # Boundary split: early-kt signs land in arena tiles aliasing
            # dead prep buffers; late kt written in place into s_tile.
            bn_all23 = smallp.tile([128, NT, NBC, 6], dt.float32, tag="bn1",
                                   name="bn_all23")

            for layer, wi, hd in ((2, 0, h2_d), (3, 1, h3_d)):
                bvec = vecs[f"b{layer}"]
                gv, bev = vecs[f"g{layer}"], vecs[f"be{layer}"]
                sigl = smallp.tile([128, NT], dt.float32, tag="sig",
                                   name=f"sig{layer}")
                bial = smallp.tile([128, NT], dt.float32, tag="bias",
                                   name=f"bia{layer}")
                hres, NRES = make_arena_h(layer)
                for ng in range(16):
                    c, l0 = (2 * ng) // 4, (2 * ng) % 4
                    for nb in range(2):
                        n = 2 * ng + nb
                        wpan = wpanp.tile([128, NT, 128], dt.float8e4,
                                          tag="wp", name="wp")
                        nc.sync.dma_start(wpan[:], ccw_out[c, wi, l0 + nb])
                        pss = [
                            psump.tile([128, BC], dt.float32, tag="ps",
                                       name=f"psl{layer}")
                            for _ in range(NBC)
                        ]
                        for kt in range(0, NT, 2):
                            for bc in range(NBC):
                                nc.tensor.matmul(
                                    pss[bc][:],
                                    wpan[:, kt : kt + 2, :],
                                    rhs(layer, kt, bc),
                                    start=(kt == 0),
                                    stop=(kt == NT - 2),
                                    perf_mode=mybir.MatmulPerfMode.DoubleRow,
                                )
                        res = hres(n - (NT - NRES)) if n >= NT - NRES \
                            else None
                        for hh in range(2):
                            ht = (res[:, hh * HB : (hh + 1) * HB]
                                  if res is not None else
                                  hqp.tile([128, HB], dt.float32,
                                           tag="hbig", name=f"ht{layer}")[:])
                            for b2 in range(2):
                                bc = hh * 2 + b2
                                nc.vector.tensor_scalar(
                                    ht[:, b2 * BC : (b2 + 1) * BC],
                                    pss[bc][:], bvec[:, n : n + 1], 0.0,
                                    op0=ALU.add, op1=ALU.max,
                                )
                                nc.vector.bn_stats(
                                    bn_all23[:, n, bc, :],
                                    ht[:, b2 * BC : (b2 + 1) * BC],
                                )
                            if res is None:
                                nc.sync.dma_start(
                                    hd[n, :, hh * HB : (hh + 1) * HB], ht
                                )
                _emit_stats_range(
                    nc, pools, bn_all23, gv, bev,
                    ccs[(layer - 1, 0)][0], ccs[(layer - 1, 0)][1],
                    0, NT, sigl, bial,
                )
                for ch in _sign_kt_closures(nc, pools, hd,
                                            lambda kt: (s_tile, kt),
                                            sigl, bial, 0, NT - NRES):
                    ch()
                for kt in range(NT - NRES, NT):
                    hsrc = hres(kt - (NT - NRES))
                    for hh in range(2):
                        nc.scalar.activation(
                            s_tile[:, kt, hh * HB : (hh + 1) * HB],
                            hsrc[:, hh * HB : (hh + 1) * HB],
                            AF.Sign,
                            bias=bial[:, kt : kt + 1],
                            scale=sigl[:, kt : kt + 1],
                        )

            # ---------- layer 4 + log_softmax ----------
            for bc in range(NBC):
                ps4 = psump.tile([16, BC], dt.float32, tag="ps", name="ps4")
                for kt in range(0, NT, 2):
                    nc.tensor.matmul(
                        ps4[:],
                        wt4[:, kt : kt + 2, :],
                        rhs(4, kt, bc),
                        start=(kt == 0),
                        stop=(kt == NT - 2),
                        perf_mode=mybir.MatmulPerfMode.DoubleRow,
                    )
                z4c = l4p.tile([OUT_C, BC], dt.float32, tag="z4", name="z4c")
                nc.scalar.activation(
                    z4c[:], ps4[0:OUT_C, :], AF.Identity, bias=b4sb[:, 0:1]
                )
                for btl in range(BC // 128):
                    bt = bc * (BC // 128) + btl
                    tp = psump.tile([128, 128], dt.float32, tag="ps",
                                    name="tpz")
                    nc.tensor.transpose(
                        tp[:], z4c[:, btl * 128 : (btl + 1) * 128],
                        id_sb[0:OUT_C, :]
                    )
                    negmx = softp.tile([128, 1], dt.float32, tag="negmx",
                                       name="negmx")
                    nc.vector.tensor_reduce(
                        negmx[:], tp[:, 0:OUT_C], axis=mybir.AxisListType.X,
                        op=ALU.max, negate=True,
                    )
                    e_sb = softp.tile([128, OUT_C], dt.float32, tag="esb",
                                      name="e_sb")
                    nc.scalar.activation(
                        e_sb[:], tp[:, 0:OUT_C], AF.Exp, bias=negmx[:, 0:1]
                    )
                    ssum = softp.tile([128, 1], dt.float32, tag="ssum",
                                      name="ssum")
                    nc.vector.tensor_reduce(
                        ssum[:], e_sb[:], axis=mybir.AxisListType.X, op=ALU.add
                    )
                    lse = softp.tile([128, 1], dt.float32, tag="lse",
                                     name="lse")
                    nc.scalar.activation(lse[:], ssum[:], AF.Ln)
                    shift = softp.tile([128, 1], dt.float32, tag="shift",
                                       name="shift")
                    nc.vector.tensor_sub(shift[:], negmx[:], lse[:])
                    outc = softp.tile([128, OUT_C], dt.float32, tag="outc",
                                      name="outc")
                    nc.scalar.activation(
                        outc[:], tp[:, 0:OUT_C], AF.Identity, bias=shift[:, 0:1]
                    )
                    nc.sync.dma_start(
                        out_p[bt * 128 : (bt + 1) * 128, :], outc[:]
                    )

    _strip_redundant_ldweights(nc)
    nc.compile()
    return nc


def _strip_redundant_ldweights(nc):
    """Delete sync-free LDWEIGHTS whose weights are already resident.

    bacc lowers each matmul into InstLdweights + non-self-loading
    InstMatmult; with bc-inner loops the same weights are reloaded 4x.
    The PE stationary array persists across (non-transpose) matmuls, so a
    repeat load with no semaphore wait/update is a pure no-op.
    """
    removed = 0
    for bb in nc.main_func.blocks:
        insts = bb.instructions
        prev_key = None
        keep = []
        for ins in insts:
            if isinstance(ins, mybir.InstLdweights):
                key = (str(ins.ins[0]) if ins.ins else None,
                       str(ins.perf_mode), str(ins.tile_position))
                if (key == prev_key and not ins.has_wait()
                        and not ins.has_update()):
                    removed += 1
                    continue
                prev_key = key
            elif ins.engine == mybir.EngineType.PE:
                if not (isinstance(ins, mybir.InstMatmult)
                        and not ins.is_transpose):
                    prev_key = None
            keep.append(ins)
        if len(keep) != len(insts):
            insts[:] = keep
    return removed


INPUT_NAMES = ["x", "W1", "b1", "g1", "be1", "W2", "b2", "g2", "be2",
               "W3", "b3", "g3", "be3", "W4", "b4"]
SHARDED = {"x", "W2", "W3"}


def _get_runner():
    """Build (once) a cached shard_map-jitted runner over the compiled NEFF.

    Mirrors concourse.bass2jax.run_bass_via_pjrt's multi-core path, but keeps
    the jitted callable so repeated calls don't re-trace/re-compile.
    """
    if "runner" in _CACHE:
        return _CACHE["runner"]
    import jax
    from jax.experimental.shard_map import shard_map
    from jax.sharding import Mesh, NamedSharding, PartitionSpec

    from concourse import bass2jax
    import concourse.mybir as mb

    if "nc" not in _CACHE:
        _CACHE["nc"] = _build()
    nc = _CACHE["nc"]
    bass2jax.install_neuronx_cc_hook()

    partition_name = (nc.partition_id_tensor.name
                      if nc.partition_id_tensor else None)
    in_names, out_names, out_avals = [], [], []
    for alloc in nc.m.functions[0].allocations:
        if not isinstance(alloc, mb.MemoryLocationSet):
            continue
        name = alloc.memorylocations[0].name
        if alloc.kind == "ExternalInput":
            if name != partition_name:
                in_names.append(name)
        elif alloc.kind == "ExternalOutput":
            out_names.append(name)
            out_avals.append(
                jax.core.ShapedArray(tuple(alloc.tensor_shape),
                                     mb.dt.np(alloc.dtype))
            )
    n_params = len(in_names)
    all_names = list(in_names) + list(out_names)
    if partition_name is not None:
        all_names.append(partition_name)

    def _body(*args):
        operands = list(args)
        if partition_name is not None:
            operands.append(bass2jax.partition_id_tensor())
        outs = bass2jax._bass_exec_p.bind(
            *operands,
            out_avals=tuple(out_avals),
            in_names=tuple(all_names),
            out_names=tuple(out_names),
            lowering_input_output_aliases=(),
            sim_require_finite=True,
            sim_require_nnan=True,
            nc=nc,
        )
        return tuple(outs)

    devices = jax.devices()[:N_CORES]
    mesh = Mesh(np.asarray(devices), ("core",))
    spec = PartitionSpec("core")
    n_outs = len(out_names)
    fn = jax.jit(
        shard_map(_body, mesh=mesh, in_specs=(spec,) * (n_params + n_outs),
                  out_specs=(spec,) * n_outs, check_rep=False),
        donate_argnums=tuple(range(n_params, n_params + n_outs)),
        keep_unused=True,
    )
    shard = NamedSharding(mesh, spec)
    out_shapes = [tuple(a.shape) for a in out_avals]
    runner = {
        "fn": fn, "in_names": in_names, "out_names": out_names,
        "out_shapes": out_shapes, "shard": shard, "jax": jax,
    }
    _CACHE["runner"] = runner
    return runner


def _device_inputs(arrs):
    r = _get_runner()
    jax = r["jax"]
    ins = []
    for name in r["in_names"]:
        if name in SHARDED:
            glob = arrs[name]
        else:
            glob = np.concatenate([arrs[name]] * N_CORES, axis=0)
        ins.append(jax.device_put(glob, r["shard"]))
    return ins


def _zero_outs():
    r = _get_runner()
    jax = r["jax"]
    return [
        jax.device_put(np.zeros((N_CORES * s[0],) + tuple(s[1:]), np.float32),
                       r["shard"])
        for s in r["out_shapes"]
    ]


def kernel(**inputs) -> np.ndarray:
    arrs = {
        k: np.ascontiguousarray(np.asarray(inputs[k], dtype=np.float32))
        for k in INPUT_NAMES
    }
    r = _get_runner()
    dev_in = _device_inputs(arrs)
    outs = r["fn"](*dev_in, *_zero_outs())
    out = np.asarray(outs[r["out_names"].index("out")])
    return out.reshape(B, OUT_C)


def bench(inputs, iters=10):
    """Steady-state execution timing with device-resident inputs."""
    import time

    arrs = {
        k: np.ascontiguousarray(np.asarray(inputs[k], dtype=np.float32))
        for k in INPUT_NAMES
    }
    r = _get_runner()
    dev_in = _device_inputs(arrs)
    fn = r["fn"]
    jax = r["jax"]
    # warmup
    jax.block_until_ready(fn(*dev_in, *_zero_outs()))
    times = []
    for _ in range(iters):
        zo = _zero_outs()
        jax.block_until_ready(dev_in)
        t0 = time.perf_counter()
        out = fn(*dev_in, *zo)
        jax.block_until_ready(out)
        times.append(time.perf_counter() - t0)
    return times
